# revision 1
# baseline (speedup 1.0000x reference)
"""Trainium2 Bass kernel for nn_DFNPureModel (retrieval_knn).

Data-parallel over batch B=8 across 8 NeuronCores; params replicated.
Per core (one batch element, S=4096 tokens, D=512, F=512, E=1024, Ne=512,
G2=10000 grid points):

  chain (fp32, feature-major):  h1 = gelu(x@W1), df = gelu(h1@W2),
           g = gelu(df@We1eff)  with  We1eff = We1[2:] + Wc@We1[:2]
           (the coords GEMM is folded into We1 on the host).
  importance^2 = ||g@We2 + be2||^2 = g^T A g + v^T g (+const, dropped since a
           constant shift never changes the ranking), with A = We2 We2^T
           precomputed on host: half the FLOPs of the ef GEMM, and the
           quadratic form's positive-sum structure makes elementwise f32r
           rounding errors cancel statistically -> A-GEMM runs single-pass
           f32r at full PE rate while keeping ~1e-5 importance accuracy
           (boundary gap at rank 512 is ~6.5e-5, so no top-k set flips).
  top-k:   kth_largest (GPSIMD, quantile with k_adj=510) -> exact 512th
           largest importance^2; mask+iota+sparse_gather -> compacted 512
           token indices; ap_gather -> selected g columns (feature-major).
           Order never matters: the output is a sum over selected entities.
  pass 2:  ef_sel = g_sel@We2 (fp32) -> positions/states (bf16)
  output side (bf16):  proj = gelu(states@Wo1p)@Wo2 with Wo1 zero-padded two
           rows so contraction aligns with ef-feature chunks;
           RBF attention factored as e = exp(20*g.p - 10*|p|^2) with the
           exp(-10*|g|^2) factor cancelled against the normalizer; the
           reference's +1e-8 becomes corr = 1e-8*exp(10*|g|^2) on the denom.

Known TRN2 hazards handled here: f32r matmul needs rounded producers and
crashes for 1<M<128 (only M=1/M=128 used); memset cannot write f32r;
ap_gather crashes on f32r dtype (reads through an F32 bitcast instead).
"""

import numpy as np
import ml_dtypes

import concourse.bass as bass
import concourse.bass_isa as bass_isa
import concourse.mybir as mybir
import concourse.tile as tile
from concourse import bacc
from concourse.bass_utils import run_bass_kernel_spmd
from concourse.masks import make_identity

F32 = mybir.dt.float32
F32R = mybir.dt.float32r
BF16 = mybir.dt.bfloat16
I16 = mybir.dt.int16
U32 = mybir.dt.uint32
AF = mybir.ActivationFunctionType
ALU = mybir.AluOpType

B, S, D, F, E, NE = 8, 4096, 512, 512, 1024, 512
EF = E + 3          # 1027
G2 = 10000
GP = 10240          # padded grid
TB = 8              # token blocks
TT = 512            # tokens per block

CHAIN = "fp32"      # "fp32" (exact) or "f32r" (fast, ~1.6e-4)


def _build(chain=CHAIN):
    nc = bacc.Bacc("TRN2", target_bir_lowering=False, debug=False,
                   enable_asserts=True, num_devices=8)

    def din(name, shape, dt):
        return nc.dram_tensor(name, list(shape), dt, kind="ExternalInput").ap()

    # aq is consumed by f32r matmuls; declaring the DRAM tensor f32r keeps
    # the producer chain f32r-clean for the BIR verifier (bits are plain f32).
    x_d = din("xT", [D, S], F32)        # host-transposed x
    a_d = din("aq", [128, 4, F], F32R)  # A = We2 We2^T (importance quadratic)
    v_d = din("vq", [128, 4], F32)      # v = 2 We2 be2
    w1_d = din("w1", [128, 4, F], F32)
    w2_d = din("w2", [128, 4, F], F32)
    # We1eff = We1[2:] + Wc @ We1[:2]  (coords GEMM folded in on host)
    we1_d = din("we1", [128, 4, F], F32)
    we2_d = din("we2", [128, 4, EF], F32)
    wo1_d = din("wo1", [128, 9, 2 * E], BF16)  # zero-padded to align sel chunks
    wo2_d = din("wo2", [128, 16, D], BF16)
    b1_d = din("b1", [128, 4], F32)
    b2_d = din("b2", [128, 4], F32)
    be1_d = din("be1", [128, 4], F32)   # be1 + bc @ We1[:2]
    be2_d = din("be2", [128, 9], F32)
    bo1_d = din("bo1", [128, 16], F32)
    bo2_d = din("bo2", [128, 4], F32)
    grid_d = din("gridT", [3, GP], BF16)
    corr_d = din("corr", [128, 80], F32)
    out_d = nc.dram_tensor("out", [G2, D], F32, kind="ExternalOutput").ap()

    CD = F32 if chain == "fp32" else F32R

    with tile.TileContext(nc) as tc:
        with tc.tile_pool(name="small", bufs=1) as small, \
             tc.tile_pool(name="keep", bufs=1) as keep:

            b1 = small.tile([128, 4], F32); nc.sync.dma_start(b1[:], b1_d[:])
            b2 = small.tile([128, 4], F32); nc.sync.dma_start(b2[:], b2_d[:])
            be1 = small.tile([128, 4], F32); nc.sync.dma_start(be1[:], be1_d[:])
            be2 = small.tile([128, 9], F32); nc.sync.dma_start(be2[:], be2_d[:])
            ident = small.tile([128, 128], F32)
            make_identity(nc, ident[:])
            ones_f32 = small.tile([128, 1], F32)
            nc.vector.memset(ones_f32[:], 1.0)
            ones_col = small.tile([128, 1], F32R)
            nc.vector.tensor_copy(ones_col[:], ones_f32[:])
            vq = small.tile([128, 4], F32)
            nc.sync.dma_start(vq[:], v_d[:])
            # importance^2 in both selection layouts, filled per token block:
            #   ipm[p, f] = imp2[t] with t = 32*p + f      (kth_largest input)
            #   iiv[p, f] = imp2[t] with t = 256*p + f     (sparse_gather input)
            ipm = small.tile([128, 32], F32)
            iiv = small.tile([16, 256], F32)

            gsel = keep.tile([128, 4, NE], F32)
            selT = keep.tile([128, 9, NE], BF16)

            with tc.tile_pool(name="wts", bufs=1) as wts:
                w1 = wts.tile([128, 4, F], CD)
                w2 = wts.tile([128, 4, F], CD)
                we1 = wts.tile([128, 4, F], CD)
                we2 = wts.tile([128, 4, EF], F32)
                aq = wts.tile([128, 4, F], F32R)
                wpairs = ((w1, w1_d), (w2, w2_d), (we1, we1_d),
                          (we2, we2_d), (aq, a_d))
                if chain == "fp32":
                    for t, d in wpairs:
                        nc.sync.dma_start(t[:], d[:])
                else:
                    with tc.tile_pool(name="wstage", bufs=2) as wst:
                        for t, d in wpairs:
                            st = wst.tile(list(t.shape), F32, tag="wst")
                            nc.sync.dma_start(st[:], d[:])
                            nc.vector.tensor_copy(t[:], st[:])

                with tc.tile_pool(name="gbuf", bufs=1) as gbuf:
                    gT = gbuf.tile([128, 4, S], F32R)

                    # ============ chain: per token block ============
                    with tc.tile_pool(name="pa", bufs=2) as pa, \
                         tc.tile_pool(name="pa1", bufs=1) as pa1, \
                         tc.tile_pool(name="mm_ps", bufs=3, space="PSUM") as mm_ps, \
                         tc.tile_pool(name="imp_ps", bufs=2, space="PSUM") as imp_ps:
                        for tb in range(TB):
                            tok = slice(tb * TT, (tb + 1) * TT)
                            xT = pa.tile([128, 4, TT], CD, tag="xT")
                            nc.sync.dma_start(
                                xT[:], x_d.rearrange("(c p) t -> p c t",
                                                     p=128)[:, :, tok])

                            h1g = pa.tile([128, 4, TT], CD, tag="h1g")
                            for m in range(4):
                                ps = mm_ps.tile([128, TT], F32, tag="mm")
                                for k in range(4):
                                    nc.tensor.matmul(
                                        ps[:], w1[:, k, m * 128:(m + 1) * 128],
                                        xT[:, k, :], start=(k == 0), stop=(k == 3))
                                nc.scalar.activation(h1g[:, m, :], ps[:], AF.Gelu,
                                                     bias=b1[:, m:m + 1])

                            dfg = pa.tile([128, 4, TT], CD, tag="dfg")
                            for m in range(4):
                                ps = mm_ps.tile([128, TT], F32, tag="mm")
                                for k in range(4):
                                    nc.tensor.matmul(
                                        ps[:], w2[:, k, m * 128:(m + 1) * 128],
                                        h1g[:, k, :], start=(k == 0), stop=(k == 3))
                                nc.scalar.activation(dfg[:, m, :], ps[:], AF.Gelu,
                                                     bias=b2[:, m:m + 1])

                            for m in range(4):
                                ps = mm_ps.tile([128, TT], F32, tag="mm")
                                for k in range(4):
                                    nc.tensor.matmul(
                                        ps[:], we1[:, k, m * 128:(m + 1) * 128],
                                        dfg[:, k, :], start=(k == 0), stop=(k == 3))
                                nc.scalar.activation(gT[:, m, tok], ps[:], AF.Gelu,
                                                     bias=be1[:, m:m + 1])

                            # importance^2 = g^T A g + v^T g  (+const, dropped:
                            # a constant shift never changes the ranking)
                            psi = imp_ps.tile([1, TT], F32, tag="psi")
                            for m in range(4):
                                ps = mm_ps.tile([128, TT], F32, tag="mm")
                                for k in range(4):
                                    nc.tensor.matmul(
                                        ps[:], aq[:, k, m * 128:(m + 1) * 128],
                                        gT[:, k, tok],
                                        start=(k == 0), stop=(k == 3))
                                prod = pa.tile([128, TT], F32R, tag="prod")
                                nc.vector.scalar_tensor_tensor(
                                    prod[:], ps[:], vq[:, m:m + 1],
                                    gT[:, m, tok].bitcast(F32),
                                    op0=ALU.add, op1=ALU.mult)
                                nc.tensor.matmul(psi[:], ones_col[:],
                                                 prod[:],
                                                 start=(m == 0), stop=(m == 3))
                            imp_tb = pa1.tile([1, TT], F32, tag="imp_tb")
                            nc.scalar.copy(imp_tb[:], psi[:])
                            # t = tb*512 + j ; ipm row = t//32, iiv row = t%16
                            nc.sync.dma_start(
                                ipm[16 * tb:16 * (tb + 1), :],
                                imp_tb[0:1, :].rearrange("p (a b) -> p a b", a=16))
                            nc.sync.dma_start(iiv[2 * tb:2 * (tb + 1), :],
                                              imp_tb[0:1, :])

                    # ============ selection ============
                    with tc.tile_pool(name="selp", bufs=1) as sp:
                        thr = sp.tile([1, 2], F32)
                        nc.gpsimd.kth_largest(thr[:], ipm[:], n_per_lane=32,
                                              k=510, quantile=1.0 - 510.5 / 4095.0)
                        iota1 = sp.tile([16, 256], F32)
                        nc.gpsimd.iota(iota1[:], pattern=[[1, 256]], base=1,
                                       channel_multiplier=256,
                                       allow_small_or_imprecise_dtypes=True)
                        thr_b = sp.tile([16, 1], F32)
                        nc.gpsimd.partition_broadcast(thr_b[:], thr[0:1, 1:2])
                        mask = sp.tile([16, 256], F32)
                        nc.vector.tensor_scalar(mask[:], iiv[:], thr_b[:], None,
                                                op0=ALU.is_ge)
                        selv = sp.tile([16, 256], F32)
                        nc.vector.scalar_tensor_tensor(
                            selv[:], mask[:], 1.0, iota1[:],
                            op0=ALU.mult, op1=ALU.mult)
                        nc.vector.tensor_scalar_add(selv[:], selv[:], -1.0)
                        sg = sp.tile([16, 32], F32)
                        nfound = sp.tile([1, 1], U32)
                        nc.gpsimd.sparse_gather(sg[:], selv[:], num_found=nfound[:])
                        idx16 = sp.tile([16, 32], I16)
                        nc.vector.tensor_copy(idx16[:], sg[:])
                        idxr = sp.tile([128, 32], I16)
                        for c in range(8):
                            nc.sync.dma_start(idxr[c * 16:(c + 1) * 16, :],
                                              idx16[:])
                        for k in range(4):
                            nc.gpsimd.ap_gather(gsel[:, k, :],
                                                gT[:, k, :].bitcast(F32),
                                                idxr[:], channels=128,
                                                num_elems=S, d=1, num_idxs=NE)
                # gbuf closed: gT freed
            # wts closed: chain weights freed

            # ============ pass 2 + output side (bf16) ============
            with tc.tile_pool(name="ob", bufs=1) as ob:
                wo2 = ob.tile([128, 16, D], BF16)
                nc.sync.dma_start(wo2[:], wo2_d[:])
                bo1 = ob.tile([128, 16], F32); nc.sync.dma_start(bo1[:], bo1_d[:])
                bo2 = ob.tile([128, 4], F32); nc.sync.dma_start(bo2[:], bo2_d[:])
                gridT = ob.tile([3, GP], BF16)
                nc.sync.dma_start(gridT[:], grid_d[:])
                corr = ob.tile([128, 80], F32); nc.sync.dma_start(corr[:], corr_d[:])

                # pass 2: ef_sel = g_sel @ We2 (fp32), We2 loaded late so its
                # DMA overlaps the GPSIMD selection work
                with tc.tile_pool(name="we2p", bufs=1) as w2p, \
                     tc.tile_pool(name="p2ps", bufs=3, space="PSUM") as p2ps:
                    we2 = w2p.tile([128, 4, EF], F32)
                    nc.sync.dma_start(we2[:], we2_d[:])
                    for fc in range(9):
                        mw = 128 if fc < 8 else 3
                        ps = p2ps.tile([128, NE], F32, tag="mm2")
                        for k in range(4):
                            nc.tensor.matmul(
                                ps[:mw, :], we2[:, k, fc * 128:fc * 128 + mw],
                                gsel[:, k, :], start=(k == 0), stop=(k == 3))
                        nc.vector.tensor_scalar(selT[:mw, fc, :], ps[:mw, :],
                                                be2[:mw, fc:fc + 1], None,
                                                op0=ALU.add)

                hT = ob.tile([128, 16, NE], BF16)
                with tc.tile_pool(name="wo1p", bufs=1) as wo1pool, \
                     tc.tile_pool(name="ops1", bufs=3, space="PSUM") as ops1:
                    wo1 = wo1pool.tile([128, 9, 2 * E], BF16)
                    nc.sync.dma_start(wo1[:], wo1_d[:])
                    for m in range(16):
                        ps = ops1.tile([128, NE], F32, tag="mmh")
                        for c in range(9):
                            mw = 128 if c < 8 else 3
                            nc.tensor.matmul(
                                ps[:], wo1[:mw, c, m * 128:(m + 1) * 128],
                                selT[:mw, c, :], start=(c == 0), stop=(c == 8))
                        nc.scalar.activation(hT[:, m, :], ps[:], AF.Gelu,
                                             bias=bo1[:, m:m + 1])

                projT = ob.tile([128, 4, NE], BF16)
                with tc.tile_pool(name="ops2", bufs=3, space="PSUM") as ops2:
                    for m in range(4):
                        ps = ops2.tile([128, NE], F32, tag="mmp")
                        for k in range(16):
                            nc.tensor.matmul(
                                ps[:], wo2[:, k, m * 128:(m + 1) * 128],
                                hT[:, k, :], start=(k == 0), stop=(k == 15))
                        nc.vector.tensor_scalar(projT[:, m, :], ps[:],
                                                bo2[:, m:m + 1], None, op0=ALU.add)
                proj = ob.tile([128, 4, D], BF16)
                ident_bf = ob.tile([128, 128], BF16)
                nc.vector.tensor_copy(ident_bf[:], ident[:])
                with tc.tile_pool(name="opst", bufs=2, space="PSUM") as opst:
                    for n4 in range(4):
                        pst = opst.tile([128, D], BF16, tag="ptp")
                        for dc in range(4):
                            nc.tensor.transpose(
                                pst[:, dc * 128:(dc + 1) * 128],
                                projT[:, dc, n4 * 128:(n4 + 1) * 128], ident_bf[:])
                        nc.vector.tensor_copy(proj[:, n4, :], pst[:])

                pos = selT[0:2, 0, :]
                paug = ob.tile([3, NE], BF16)
                nc.vector.tensor_scalar_mul(paug[0:2, :], pos, 20.0)
                possq = ob.tile([2, NE], F32)
                nc.scalar.activation(possq[:], pos, AF.Square)
                p2s = ob.tile([2, NE], F32)
                nc.gpsimd.partition_all_reduce(p2s[:], possq[:], channels=2,
                                               reduce_op=bass_isa.ReduceOp.add)
                p2bf = ob.tile([1, NE], BF16)
                nc.vector.tensor_scalar_mul(p2bf[:], p2s[0:1, :], -10.0)
                nc.sync.dma_start(paug[2:3, :], p2bf[:])

                eT = ob.tile([128, 4, GP], BF16)
                ones_bf = ob.tile([128, 1], BF16)
                nc.vector.memset(ones_bf[:], 1.0)
                den_pm = ob.tile([128, 80], F32)
                rec = ob.tile([128, 80], F32)
                # single PSUM scope: exp, denom and attn-out interleave
                with tc.tile_pool(name="eps", bufs=2, space="PSUM") as eps, \
                     tc.tile_pool(name="dps", bufs=2, space="PSUM") as dps, \
                     tc.tile_pool(name="oops", bufs=2, space="PSUM") as oops, \
                     tc.tile_pool(name="dsb", bufs=3) as dsb, \
                     tc.tile_pool(name="oo", bufs=3) as oo:
                    # gb-outer so each grid block's 4 n-chunks finish together
                    for gb in range(10):
                        for n4 in range(4):
                            pse = eps.tile([128, 1024], F32, tag="pse")
                            for gs in range(2):
                                g0 = gb * 1024 + gs * 512
                                nc.tensor.matmul(
                                    pse[:, gs * 512:(gs + 1) * 512],
                                    paug[:, n4 * 128:(n4 + 1) * 128],
                                    gridT[:, g0:g0 + 512],
                                    start=True, stop=True)
                            nc.scalar.activation(
                                eT[:, n4, gb * 1024:(gb + 1) * 1024],
                                pse[:], AF.Exp)
                        for gh in range(2):
                            g5 = gb * 2 + gh
                            psd = dps.tile([1, 512], F32, tag="psd")
                            for n4 in range(4):
                                nc.tensor.matmul(
                                    psd[:], ones_bf[:],
                                    eT[:, n4, g5 * 512:(g5 + 1) * 512],
                                    start=(n4 == 0), stop=(n4 == 3))
                            den_tb = dsb.tile([1, 512], F32, tag="den_tb")
                            nc.scalar.copy(den_tb[:], psd[:])
                            # den_pm[p, c] = den[128*c + p]; c = 4*g5..4*g5+3
                            for c4 in range(4):
                                nc.sync.dma_start(
                                    den_pm[:, 4 * g5 + c4:4 * g5 + c4 + 1],
                                    den_tb[0:1, c4 * 128:(c4 + 1) * 128])
                            # per-chunk recip: attn scaling never waits globally
                            sl = slice(4 * g5, 4 * (g5 + 1))
                            nc.vector.tensor_add(den_pm[:, sl], den_pm[:, sl],
                                                 corr[:, sl])
                            nc.vector.reciprocal(rec[:, sl], den_pm[:, sl])

                    for gc in range(79):
                        rows = 128 if gc < 78 else 16
                        ps = oops.tile([128, D], F32, tag="mmo")
                        for k in range(4):
                            nc.tensor.matmul(
                                ps[:rows, :],
                                eT[:, k, gc * 128:gc * 128 + rows],
                                proj[:, k, :], start=(k == 0), stop=(k == 3))
                        ot = oo.tile([128, D], F32, tag="ot")
                        nc.vector.tensor_scalar(ot[:rows, :], ps[:rows, :],
                                                rec[:rows, gc:gc + 1], None,
                                                op0=ALU.mult)
                        nc.sync.dma_start(out_d[gc * 128:gc * 128 + rows, :],
                                          ot[:rows, :])
    nc.compile()
    return nc


_NC_CACHE = {}


def _host_inputs(inputs):
    """Replicated host-side tensor prep (layout shuffles only)."""
    f32 = np.float32
    bf = ml_dtypes.bfloat16
    W1 = np.asarray(inputs["W1"], f32)
    W2 = np.asarray(inputs["W2"], f32)
    Wc = np.asarray(inputs["Wc"], f32)
    We1 = np.asarray(inputs["We1"], f32)
    We2 = np.asarray(inputs["We2"], f32)
    Wo1 = np.asarray(inputs["Wo1"], f32)
    Wo2 = np.asarray(inputs["Wo2"], f32)
    b1 = np.asarray(inputs["b1"], f32); b2 = np.asarray(inputs["b2"], f32)
    bc = np.asarray(inputs["bc"], f32); be1 = np.asarray(inputs["be1"], f32)
    be2 = np.asarray(inputs["be2"], f32)
    bo1 = np.asarray(inputs["bo1"], f32); bo2 = np.asarray(inputs["bo2"], f32)

    def kchunk(w, nk):   # [K, N] -> [128, nk, N]
        return np.ascontiguousarray(
            w.reshape(nk, 128, w.shape[1]).transpose(1, 0, 2))

    def bvec(b, ncol):   # [N] -> [128, ncol]
        return np.ascontiguousarray(b.reshape(ncol, 128).T)

    # fold coords GEMM: fi@We1 = df@(We1[2:] + Wc@We1[:2]) + (be1 + bc@We1[:2])
    We1_64 = We1.astype(np.float64)
    we1_eff = (We1_64[2:] + Wc.astype(np.float64) @ We1_64[:2]).astype(f32)
    be1_eff = (be1.astype(np.float64)
               + bc.astype(np.float64) @ We1_64[:2]).astype(f32)
    wo1p = np.zeros((9 * 128, 2 * E), f32)
    wo1p[2:2 + E] = Wo1

    We2_64 = We2.astype(np.float64)
    Aq = (We2_64 @ We2_64.T).astype(f32)               # [512, 512]
    vq = (2.0 * (We2_64 @ be2.astype(np.float64))).astype(f32)  # [512]

    g = np.linspace(-1.0, 1.0, 100, dtype=f32)
    gx, gy = np.meshgrid(g, g, indexing="ij")
    grid = np.stack([gx.ravel(), gy.ravel()], -1).astype(f32)
    g2 = (grid * grid).sum(-1)
    gridT = np.zeros((3, GP), f32)
    gridT[0, :G2] = grid[:, 0]
    gridT[1, :G2] = grid[:, 1]
    gridT[2, :G2] = 1.0
    corr_full = np.ones(GP, f32)
    corr_full[:G2] = (1e-8 * np.exp(10.0 * g2.astype(np.float64))).astype(f32)
    corr = np.ascontiguousarray(corr_full.reshape(80, 128).T)

    be2p = np.zeros(9 * 128, f32); be2p[:EF] = be2

    return {
        "aq": kchunk(Aq, 4), "vq": bvec(vq, 4),
        "w1": kchunk(W1, 4), "w2": kchunk(W2, 4),
        "we1": kchunk(we1_eff, 4),
        "we2": kchunk(We2, 4),
        "wo1": kchunk(wo1p, 9).astype(bf),
        "wo2": kchunk(Wo2, 16).astype(bf),
        "b1": bvec(b1, 4), "b2": bvec(b2, 4),
        "be1": bvec(be1_eff, 4), "be2": bvec(be2p, 9),
        "bo1": bvec(bo1, 16), "bo2": bvec(bo2, 4),
        "gridT": gridT.astype(bf), "corr": corr,
    }


def kernel(**inputs):
    if CHAIN not in _NC_CACHE:
        _NC_CACHE[CHAIN] = _build(CHAIN)
    nc = _NC_CACHE[CHAIN]
    shared = _host_inputs(inputs)
    x = np.asarray(inputs["x"], np.float32)
    in_maps = []
    for b in range(B):
        m = dict(shared)
        m["xT"] = np.ascontiguousarray(x[b].T)
        in_maps.append(m)
    res = run_bass_kernel_spmd(nc, in_maps, core_ids=list(range(B)))
    return np.stack([r["out"] for r in res.results]).astype(np.float32)



# revision 4
# speedup vs baseline: 1.5864x; 1.5864x over previous
"""Trainium2 Bass kernel for nn_DFNPureModel (retrieval_knn).

Data-parallel over batch B=8 across 8 NeuronCores; params replicated.
Per core (one batch element, S=4096 tokens, D=512, F=512, E=1024, Ne=512,
G2=10000 grid points):

  chain (fp32, feature-major):  h1 = gelu(x@W1), df = gelu(h1@W2),
           g = gelu(df@We1eff)  with  We1eff = We1[2:] + Wc@We1[:2]
           (the coords GEMM is folded into We1 on the host).
  importance^2 = ||g@We2 + be2||^2 = g^T A g + v^T g (+const, dropped since a
           constant shift never changes the ranking), with A = We2 We2^T
           precomputed on host: half the FLOPs of the ef GEMM, and the
           quadratic form's positive-sum structure makes elementwise f32r
           rounding errors cancel statistically -> A-GEMM runs single-pass
           f32r at full PE rate while keeping ~1e-5 importance accuracy
           (boundary gap at rank 512 is ~6.5e-5, so no top-k set flips).
  top-k:   kth_largest (GPSIMD, quantile with k_adj=510) -> exact 512th
           largest importance^2; mask+iota+sparse_gather -> compacted 512
           token indices; ap_gather -> selected g columns (feature-major).
           Order never matters: the output is a sum over selected entities.
  pass 2:  ef_sel = g_sel@We2 (fp32) -> positions/states (bf16)
  output side (bf16):  proj = gelu(states@Wo1p)@Wo2 with Wo1 zero-padded two
           rows so contraction aligns with ef-feature chunks;
           RBF attention factored as e = exp(20*g.p - 10*|p|^2) with the
           exp(-10*|g|^2) factor cancelled against the normalizer; the
           reference's +1e-8 becomes corr = 1e-8*exp(10*|g|^2) on the denom.

Known TRN2 hazards handled here: f32r matmul needs rounded producers and
crashes for 1<M<128 (only M=1/M=128 used); memset cannot write f32r;
ap_gather crashes on f32r dtype (reads through an F32 bitcast instead).
"""

import numpy as np
import ml_dtypes

import concourse.bass as bass
import concourse.bass_isa as bass_isa
import concourse.mybir as mybir
import concourse.tile as tile
from concourse import bacc
from concourse.bass_utils import run_bass_kernel_spmd
from concourse.masks import make_identity

F32 = mybir.dt.float32
F32R = mybir.dt.float32r
BF16 = mybir.dt.bfloat16
I16 = mybir.dt.int16
U32 = mybir.dt.uint32
AF = mybir.ActivationFunctionType
ALU = mybir.AluOpType

B, S, D, F, E, NE = 8, 4096, 512, 512, 1024, 512
EF = E + 3          # 1027
G2 = 10000
GP = 10240          # padded grid
TB = 8              # token blocks
TT = 512            # tokens per block

CHAIN = "fp32"      # "fp32" (exact) or "f32r" (fast, ~1.6e-4)


def _build(chain=CHAIN):
    nc = bacc.Bacc("TRN2", target_bir_lowering=False, debug=False,
                   enable_asserts=True, num_devices=8)

    def din(name, shape, dt):
        return nc.dram_tensor(name, list(shape), dt, kind="ExternalInput").ap()

    CD = F32 if chain == "fp32" else F32R

    # f32r-consumed tensors are declared f32r in DRAM (bits are plain f32):
    # DMA straight into an f32r tile keeps the producer chain f32r-clean for
    # the BIR verifier without a casting copy.
    x_d = din("xT", [D, S], CD)         # host-transposed x
    a_d = din("aq", [128, 4, F], F32R)  # A = We2 We2^T (importance quadratic)
    v_d = din("vq", [128, 4], F32)      # v = 2 We2 be2
    w1_d = din("w1", [128, 4, F], CD)
    w2_d = din("w2", [128, 4, F], CD)
    # We1eff = We1[2:] + Wc @ We1[:2]  (coords GEMM folded in on host)
    we1_d = din("we1", [128, 4, F], CD)
    we2_d = din("we2", [128, 4, EF], F32)
    wo1_d = din("wo1", [128, 9, 2 * E], BF16)  # zero-padded to align sel chunks
    wo2_d = din("wo2", [128, 16, D], BF16)
    b1_d = din("b1", [128, 4], F32)
    b2_d = din("b2", [128, 4], F32)
    be1_d = din("be1", [128, 4], F32)   # be1 + bc @ We1[:2]
    be2_d = din("be2", [128, 9], F32)
    bo1_d = din("bo1", [128, 16], F32)
    bo2_d = din("bo2", [128, 4], F32)
    grid_d = din("gridT", [3, GP], BF16)
    corr_d = din("corr", [128, 80], F32)
    out_d = nc.dram_tensor("out", [G2, D], F32, kind="ExternalOutput").ap()

    with tile.TileContext(nc) as tc:
        with tc.tile_pool(name="small", bufs=1) as small, \
             tc.tile_pool(name="keep", bufs=1) as keep:

            b1 = small.tile([128, 4], F32); nc.sync.dma_start(b1[:], b1_d[:])
            b2 = small.tile([128, 4], F32); nc.sync.dma_start(b2[:], b2_d[:])
            be1 = small.tile([128, 4], F32); nc.sync.dma_start(be1[:], be1_d[:])
            be2 = small.tile([128, 9], F32); nc.sync.dma_start(be2[:], be2_d[:])
            ident = small.tile([128, 128], F32)
            make_identity(nc, ident[:])
            ones_f32 = small.tile([128, 1], F32)
            nc.vector.memset(ones_f32[:], 1.0)
            ones_col = small.tile([128, 1], F32R)
            nc.vector.tensor_copy(ones_col[:], ones_f32[:])
            vq = small.tile([128, 4], F32)
            nc.sync.dma_start(vq[:], v_d[:])
            # importance^2 in both selection layouts, filled per token block:
            #   ipm[p, f] = imp2[t] with t = 32*p + f      (kth_largest input)
            #   iiv[p, f] = imp2[t] with t = 256*p + f     (sparse_gather input)
            ipm = small.tile([128, 32], F32)
            iiv = small.tile([16, 256], F32)

            gsel = keep.tile([128, 4, NE], F32)
            selT = keep.tile([128, 9, NE], BF16)

            with tc.tile_pool(name="wts", bufs=1) as wts:
                w1 = wts.tile([128, 4, F], CD)
                w2 = wts.tile([128, 4, F], CD)
                we1 = wts.tile([128, 4, F], CD)
                we2 = wts.tile([128, 4, EF], F32)
                aq = wts.tile([128, 4, F], F32R)
                wpairs = ((w1, w1_d), (w2, w2_d), (we1, we1_d),
                          (we2, we2_d), (aq, a_d))
                for t, d in wpairs:
                    nc.sync.dma_start(t[:], d[:])

                with tc.tile_pool(name="gbuf", bufs=1) as gbuf:
                    gT = gbuf.tile([128, 4, S], F32R)

                    # ============ chain: per token block ============
                    with tc.tile_pool(name="pa", bufs=2) as pa, \
                         tc.tile_pool(name="pa1", bufs=1) as pa1, \
                         tc.tile_pool(name="mm_ps", bufs=3, space="PSUM") as mm_ps, \
                         tc.tile_pool(name="imp_ps", bufs=2, space="PSUM") as imp_ps:
                        for tb in range(TB):
                            tok = slice(tb * TT, (tb + 1) * TT)
                            xT = pa.tile([128, 4, TT], CD, tag="xT")
                            nc.sync.dma_start(
                                xT[:], x_d.rearrange("(c p) t -> p c t",
                                                     p=128)[:, :, tok])

                            h1g = pa.tile([128, 4, TT], CD, tag="h1g")
                            for m in range(4):
                                ps = mm_ps.tile([128, TT], F32, tag="mm")
                                for k in range(4):
                                    nc.tensor.matmul(
                                        ps[:], w1[:, k, m * 128:(m + 1) * 128],
                                        xT[:, k, :], start=(k == 0), stop=(k == 3))
                                nc.scalar.activation(h1g[:, m, :], ps[:], AF.Gelu,
                                                     bias=b1[:, m:m + 1])

                            dfg = pa.tile([128, 4, TT], CD, tag="dfg")
                            for m in range(4):
                                ps = mm_ps.tile([128, TT], F32, tag="mm")
                                for k in range(4):
                                    nc.tensor.matmul(
                                        ps[:], w2[:, k, m * 128:(m + 1) * 128],
                                        h1g[:, k, :], start=(k == 0), stop=(k == 3))
                                nc.scalar.activation(dfg[:, m, :], ps[:], AF.Gelu,
                                                     bias=b2[:, m:m + 1])

                            for m in range(4):
                                ps = mm_ps.tile([128, TT], F32, tag="mm")
                                for k in range(4):
                                    nc.tensor.matmul(
                                        ps[:], we1[:, k, m * 128:(m + 1) * 128],
                                        dfg[:, k, :], start=(k == 0), stop=(k == 3))
                                nc.scalar.activation(gT[:, m, tok], ps[:], AF.Gelu,
                                                     bias=be1[:, m:m + 1])

                            # importance^2 = g^T A g + v^T g  (+const, dropped:
                            # a constant shift never changes the ranking)
                            psi = imp_ps.tile([1, TT], F32, tag="psi")
                            for m in range(4):
                                ps = mm_ps.tile([128, TT], F32, tag="mm")
                                for k in range(4):
                                    nc.tensor.matmul(
                                        ps[:], aq[:, k, m * 128:(m + 1) * 128],
                                        gT[:, k, tok],
                                        start=(k == 0), stop=(k == 3))
                                prod = pa.tile([128, TT], F32R, tag="prod")
                                nc.vector.scalar_tensor_tensor(
                                    prod[:], ps[:], vq[:, m:m + 1],
                                    gT[:, m, tok].bitcast(F32),
                                    op0=ALU.add, op1=ALU.mult)
                                nc.tensor.matmul(psi[:], ones_col[:],
                                                 prod[:],
                                                 start=(m == 0), stop=(m == 3))
                            imp_tb = pa1.tile([1, TT], F32, tag="imp_tb")
                            nc.scalar.copy(imp_tb[:], psi[:])
                            # t = tb*512 + j ; ipm row = t//32, iiv row = t%16
                            nc.sync.dma_start(
                                ipm[16 * tb:16 * (tb + 1), :],
                                imp_tb[0:1, :].rearrange("p (a b) -> p a b", a=16))
                            nc.sync.dma_start(iiv[2 * tb:2 * (tb + 1), :],
                                              imp_tb[0:1, :])

                    # ============ selection ============
                    with tc.tile_pool(name="selp", bufs=1) as sp:
                        thr = sp.tile([1, 2], F32)
                        nc.gpsimd.kth_largest(thr[:], ipm[:], n_per_lane=32,
                                              k=510, quantile=1.0 - 510.5 / 4095.0)
                        iota1 = sp.tile([16, 256], F32)
                        nc.gpsimd.iota(iota1[:], pattern=[[1, 256]], base=1,
                                       channel_multiplier=256,
                                       allow_small_or_imprecise_dtypes=True)
                        thr_b = sp.tile([16, 1], F32)
                        nc.gpsimd.partition_broadcast(thr_b[:], thr[0:1, 1:2])
                        mask = sp.tile([16, 256], F32)
                        nc.vector.tensor_scalar(mask[:], iiv[:], thr_b[:], None,
                                                op0=ALU.is_ge)
                        selv = sp.tile([16, 256], F32)
                        nc.vector.scalar_tensor_tensor(
                            selv[:], mask[:], 1.0, iota1[:],
                            op0=ALU.mult, op1=ALU.mult)
                        nc.vector.tensor_scalar_add(selv[:], selv[:], -1.0)
                        sg = sp.tile([16, 32], F32)
                        nfound = sp.tile([1, 1], U32)
                        nc.gpsimd.sparse_gather(sg[:], selv[:], num_found=nfound[:])
                        idx16 = sp.tile([16, 32], I16)
                        nc.vector.tensor_copy(idx16[:], sg[:])
                        idxr = sp.tile([128, 32], I16)
                        for c in range(8):
                            nc.sync.dma_start(idxr[c * 16:(c + 1) * 16, :],
                                              idx16[:])
                        for k in range(4):
                            nc.gpsimd.ap_gather(gsel[:, k, :],
                                                gT[:, k, :].bitcast(F32),
                                                idxr[:], channels=128,
                                                num_elems=S, d=1, num_idxs=NE)
                # gbuf closed: gT freed
            # wts closed: chain weights freed

            # ============ pass 2 + output side (bf16) ============
            with tc.tile_pool(name="ob", bufs=1) as ob:
                wo2 = ob.tile([128, 16, D], BF16)
                nc.sync.dma_start(wo2[:], wo2_d[:])
                bo1 = ob.tile([128, 16], F32); nc.sync.dma_start(bo1[:], bo1_d[:])
                bo2 = ob.tile([128, 4], F32); nc.sync.dma_start(bo2[:], bo2_d[:])
                gridT = ob.tile([3, GP], BF16)
                nc.sync.dma_start(gridT[:], grid_d[:])
                corr = ob.tile([128, 80], F32); nc.sync.dma_start(corr[:], corr_d[:])

                # pass 2: ef_sel = g_sel @ We2 (fp32), We2 loaded late so its
                # DMA overlaps the GPSIMD selection work
                with tc.tile_pool(name="we2p", bufs=1) as w2p, \
                     tc.tile_pool(name="p2ps", bufs=3, space="PSUM") as p2ps:
                    we2 = w2p.tile([128, 4, EF], F32)
                    nc.sync.dma_start(we2[:], we2_d[:])
                    for fc in range(9):
                        mw = 128 if fc < 8 else 3
                        ps = p2ps.tile([128, NE], F32, tag="mm2")
                        for k in range(4):
                            nc.tensor.matmul(
                                ps[:mw, :], we2[:, k, fc * 128:fc * 128 + mw],
                                gsel[:, k, :], start=(k == 0), stop=(k == 3))
                        nc.vector.tensor_scalar(selT[:mw, fc, :], ps[:mw, :],
                                                be2[:mw, fc:fc + 1], None,
                                                op0=ALU.add)

                hT = ob.tile([128, 16, NE], BF16)
                with tc.tile_pool(name="wo1p", bufs=1) as wo1pool, \
                     tc.tile_pool(name="ops1", bufs=3, space="PSUM") as ops1:
                    wo1 = wo1pool.tile([128, 9, 2 * E], BF16)
                    nc.sync.dma_start(wo1[:], wo1_d[:])
                    for m in range(16):
                        ps = ops1.tile([128, NE], F32, tag="mmh")
                        for c in range(9):
                            mw = 128 if c < 8 else 3
                            nc.tensor.matmul(
                                ps[:], wo1[:mw, c, m * 128:(m + 1) * 128],
                                selT[:mw, c, :], start=(c == 0), stop=(c == 8))
                        nc.scalar.activation(hT[:, m, :], ps[:], AF.Gelu,
                                             bias=bo1[:, m:m + 1])

                projT = ob.tile([128, 4, NE], BF16)
                with tc.tile_pool(name="ops2", bufs=3, space="PSUM") as ops2:
                    for m in range(4):
                        ps = ops2.tile([128, NE], F32, tag="mmp")
                        for k in range(16):
                            nc.tensor.matmul(
                                ps[:], wo2[:, k, m * 128:(m + 1) * 128],
                                hT[:, k, :], start=(k == 0), stop=(k == 15))
                        nc.vector.tensor_scalar(projT[:, m, :], ps[:],
                                                bo2[:, m:m + 1], None, op0=ALU.add)
                proj = ob.tile([128, 4, D], BF16)
                ident_bf = ob.tile([128, 128], BF16)
                nc.vector.tensor_copy(ident_bf[:], ident[:])
                with tc.tile_pool(name="opst", bufs=2, space="PSUM") as opst:
                    for n4 in range(4):
                        pst = opst.tile([128, D], BF16, tag="ptp")
                        for dc in range(4):
                            nc.tensor.transpose(
                                pst[:, dc * 128:(dc + 1) * 128],
                                projT[:, dc, n4 * 128:(n4 + 1) * 128], ident_bf[:])
                        nc.vector.tensor_copy(proj[:, n4, :], pst[:])

                pos = selT[0:2, 0, :]
                paug = ob.tile([3, NE], BF16)
                nc.vector.tensor_scalar_mul(paug[0:2, :], pos, 20.0)
                possq = ob.tile([2, NE], F32)
                nc.scalar.activation(possq[:], pos, AF.Square)
                p2s = ob.tile([2, NE], F32)
                nc.gpsimd.partition_all_reduce(p2s[:], possq[:], channels=2,
                                               reduce_op=bass_isa.ReduceOp.add)
                p2bf = ob.tile([1, NE], BF16)
                nc.vector.tensor_scalar_mul(p2bf[:], p2s[0:1, :], -10.0)
                nc.sync.dma_start(paug[2:3, :], p2bf[:])

                eT = ob.tile([128, 4, GP], BF16)
                ones_bf = ob.tile([128, 1], BF16)
                nc.vector.memset(ones_bf[:], 1.0)
                den_pm = ob.tile([128, 80], F32)
                rec = ob.tile([128, 80], F32)
                # single PSUM scope: exp, denom and attn-out interleave
                with tc.tile_pool(name="eps", bufs=2, space="PSUM") as eps, \
                     tc.tile_pool(name="dps", bufs=2, space="PSUM") as dps, \
                     tc.tile_pool(name="oops", bufs=2, space="PSUM") as oops, \
                     tc.tile_pool(name="dsb", bufs=3) as dsb, \
                     tc.tile_pool(name="oo", bufs=3) as oo:
                    # gb-outer so each grid block's 4 n-chunks finish together
                    for gb in range(10):
                        for n4 in range(4):
                            pse = eps.tile([128, 1024], F32, tag="pse")
                            for gs in range(2):
                                g0 = gb * 1024 + gs * 512
                                nc.tensor.matmul(
                                    pse[:, gs * 512:(gs + 1) * 512],
                                    paug[:, n4 * 128:(n4 + 1) * 128],
                                    gridT[:, g0:g0 + 512],
                                    start=True, stop=True)
                            nc.scalar.activation(
                                eT[:, n4, gb * 1024:(gb + 1) * 1024],
                                pse[:], AF.Exp)
                        for gh in range(2):
                            g5 = gb * 2 + gh
                            psd = dps.tile([1, 512], F32, tag="psd")
                            for n4 in range(4):
                                nc.tensor.matmul(
                                    psd[:], ones_bf[:],
                                    eT[:, n4, g5 * 512:(g5 + 1) * 512],
                                    start=(n4 == 0), stop=(n4 == 3))
                            den_tb = dsb.tile([1, 512], F32, tag="den_tb")
                            nc.scalar.copy(den_tb[:], psd[:])
                            # den_pm[p, c] = den[128*c + p]; c = 4*g5..4*g5+3
                            for c4 in range(4):
                                nc.sync.dma_start(
                                    den_pm[:, 4 * g5 + c4:4 * g5 + c4 + 1],
                                    den_tb[0:1, c4 * 128:(c4 + 1) * 128])
                            # per-chunk recip: attn scaling never waits globally
                            sl = slice(4 * g5, 4 * (g5 + 1))
                            nc.vector.tensor_add(den_pm[:, sl], den_pm[:, sl],
                                                 corr[:, sl])
                            nc.vector.reciprocal(rec[:, sl], den_pm[:, sl])

                    for gc in range(79):
                        rows = 128 if gc < 78 else 16
                        ps = oops.tile([128, D], F32, tag="mmo")
                        for k in range(4):
                            nc.tensor.matmul(
                                ps[:rows, :],
                                eT[:, k, gc * 128:gc * 128 + rows],
                                proj[:, k, :], start=(k == 0), stop=(k == 3))
                        ot = oo.tile([128, D], F32, tag="ot")
                        nc.vector.tensor_scalar(ot[:rows, :], ps[:rows, :],
                                                rec[:rows, gc:gc + 1], None,
                                                op0=ALU.mult)
                        nc.sync.dma_start(out_d[gc * 128:gc * 128 + rows, :],
                                          ot[:rows, :])
    nc.compile()
    return nc


_NC_CACHE = {}


def _host_inputs(inputs):
    """Replicated host-side tensor prep (layout shuffles only)."""
    f32 = np.float32
    bf = ml_dtypes.bfloat16
    W1 = np.asarray(inputs["W1"], f32)
    W2 = np.asarray(inputs["W2"], f32)
    Wc = np.asarray(inputs["Wc"], f32)
    We1 = np.asarray(inputs["We1"], f32)
    We2 = np.asarray(inputs["We2"], f32)
    Wo1 = np.asarray(inputs["Wo1"], f32)
    Wo2 = np.asarray(inputs["Wo2"], f32)
    b1 = np.asarray(inputs["b1"], f32); b2 = np.asarray(inputs["b2"], f32)
    bc = np.asarray(inputs["bc"], f32); be1 = np.asarray(inputs["be1"], f32)
    be2 = np.asarray(inputs["be2"], f32)
    bo1 = np.asarray(inputs["bo1"], f32); bo2 = np.asarray(inputs["bo2"], f32)

    def kchunk(w, nk):   # [K, N] -> [128, nk, N]
        return np.ascontiguousarray(
            w.reshape(nk, 128, w.shape[1]).transpose(1, 0, 2))

    def bvec(b, ncol):   # [N] -> [128, ncol]
        return np.ascontiguousarray(b.reshape(ncol, 128).T)

    # fold coords GEMM: fi@We1 = df@(We1[2:] + Wc@We1[:2]) + (be1 + bc@We1[:2])
    We1_64 = We1.astype(np.float64)
    we1_eff = (We1_64[2:] + Wc.astype(np.float64) @ We1_64[:2]).astype(f32)
    be1_eff = (be1.astype(np.float64)
               + bc.astype(np.float64) @ We1_64[:2]).astype(f32)
    wo1p = np.zeros((9 * 128, 2 * E), f32)
    wo1p[2:2 + E] = Wo1

    We2_64 = We2.astype(np.float64)
    Aq = (We2_64 @ We2_64.T).astype(f32)               # [512, 512]
    vq = (2.0 * (We2_64 @ be2.astype(np.float64))).astype(f32)  # [512]

    g = np.linspace(-1.0, 1.0, 100, dtype=f32)
    gx, gy = np.meshgrid(g, g, indexing="ij")
    grid = np.stack([gx.ravel(), gy.ravel()], -1).astype(f32)
    g2 = (grid * grid).sum(-1)
    gridT = np.zeros((3, GP), f32)
    gridT[0, :G2] = grid[:, 0]
    gridT[1, :G2] = grid[:, 1]
    gridT[2, :G2] = 1.0
    corr_full = np.ones(GP, f32)
    corr_full[:G2] = (1e-8 * np.exp(10.0 * g2.astype(np.float64))).astype(f32)
    corr = np.ascontiguousarray(corr_full.reshape(80, 128).T)

    be2p = np.zeros(9 * 128, f32); be2p[:EF] = be2

    return {
        "aq": kchunk(Aq, 4), "vq": bvec(vq, 4),
        "w1": kchunk(W1, 4), "w2": kchunk(W2, 4),
        "we1": kchunk(we1_eff, 4),
        "we2": kchunk(We2, 4),
        "wo1": kchunk(wo1p, 9).astype(bf),
        "wo2": kchunk(Wo2, 16).astype(bf),
        "b1": bvec(b1, 4), "b2": bvec(b2, 4),
        "be1": bvec(be1_eff, 4), "be2": bvec(be2p, 9),
        "bo1": bvec(bo1, 16), "bo2": bvec(bo2, 4),
        "gridT": gridT.astype(bf), "corr": corr,
    }


def kernel(**inputs):
    if CHAIN not in _NC_CACHE:
        _NC_CACHE[CHAIN] = _build(CHAIN)
    nc = _NC_CACHE[CHAIN]
    shared = _host_inputs(inputs)
    x = np.asarray(inputs["x"], np.float32)
    in_maps = []
    for b in range(B):
        m = dict(shared)
        m["xT"] = np.ascontiguousarray(x[b].T)
        in_maps.append(m)
    res = run_bass_kernel_spmd(nc, in_maps, core_ids=list(range(B)))
    return np.stack([r["out"] for r in res.results]).astype(np.float32)

